# revision 9
# baseline (speedup 1.0000x reference)
"""AttnNet kernel for Trainium2: attn = softmax(einsum("bsh,bh->bs", facts, questions))[:, None, :].

Full shapes: questions [64, 4096] f32, facts [64, 512, 4096] f32 -> out [64, 1, 512] f32.
Data-parallel over batch: 8 batches per NeuronCore x 8 cores, no collectives.

Per-core dataflow (B_LOC=8, S=512, H=4096):
  - facts streamed as 32 contiguous [128(s), 4096(h)] tiles (2 MiB each), DMA
    instructions alternating between the two HWDGE rings (sync + scalar) so DGE
    setup on one ring overlaps transfers on the other (keeps all 16 DMA engines fed).
  - questions loaded once as a flat [1, 8*4096] tile on partition 0; q[b]
    broadcast to 128 partitions via gpsimd partition_broadcast per batch.
  - Fused DVE tensor_tensor_reduce: (facts_tile * q_b) with row-sum accumulate
    -> energies column E[:, b*4+c] in one pass (no separate ACT pass).
  - Epilogue: PE-transpose E [128,32] -> [32,128] (PSUM), ACT copy to SBUF,
    regroup to [8, 512] via SBUF->SBUF DMA, then softmax: -max (DVE), fused
    exp+sum (ACT), reciprocal + scale (DVE), DMA out.
"""

import numpy as np

B, S, H = 64, 512, 4096
N_CORES = 8
B_LOC = B // N_CORES  # 8
P = 128
SC = S // P  # 4 s-chunks per batch

_CACHE = {}


def _build_bass():
    import concourse.bacc as bacc
    import concourse.mybir as mybir
    import concourse.tile as tile
    from concourse.masks import make_identity

    f32 = mybir.dt.float32

    nc = bacc.Bacc("TRN2", target_bir_lowering=False, debug=False)
    facts = nc.dram_tensor("facts", [B_LOC, S, H], f32, kind="ExternalInput").ap()
    questions = nc.dram_tensor("questions", [B_LOC, H], f32, kind="ExternalInput").ap()
    attn = nc.dram_tensor("attn", [B_LOC, S], f32, kind="ExternalOutput").ap()

    with tile.TileContext(nc) as tc:
        with (
            tc.tile_pool(name="consts", bufs=1) as consts,
            tc.tile_pool(name="fpool", bufs=6) as fpool,
            tc.tile_pool(name="qrow", bufs=2) as qrow,
            tc.tile_pool(name="qsb", bufs=2) as qsb,
            tc.tile_pool(name="pq", bufs=1, space="PSUM") as pqpool,
        ):
            identity = consts.tile([P, P], f32)
            make_identity(nc, identity[:])



            # energies, column b*SC+c holds energies[b, c*128:(c+1)*128] on partitions
            E = consts.tile([P, B_LOC * SC], f32)
            # DVE dump target for the fused multiply+reduce (value unused)
            scratch = consts.tile([P, H], f32)

            qrings = [nc.sync, nc.scalar]
            for b in range(B_LOC):
                q_row = qrow.tile([1, H], f32)
                qrings[b % 2].dma_start(out=q_row[:], in_=questions[b : b + 1, :])
                q_b = qsb.tile([P, H], f32)
                nc.gpsimd.partition_broadcast(q_b[:], q_row[:])

                for c in range(SC):
                    ftile = fpool.tile([P, H], f32)
                    k = b * SC + c
                    qrings[k % 2].dma_start(
                        out=ftile[:], in_=facts[b, c * P : (c + 1) * P, :]
                    )
                    nc.vector.tensor_mul(out=scratch[:], in0=ftile[:], in1=q_b[:])
                    nc.scalar.activation(
                        scratch[:],
                        scratch[:],
                        mybir.ActivationFunctionType.Copy,
                        accum_out=E[:, k : k + 1],
                    )

            # --- softmax epilogue ---
            # transpose E [128, 32] -> [32, 128] (PSUM), copy to SBUF, then regroup
            # [32, 128] (p = b*4+c) -> [8, 512]: both traverse in (b, c, i) order
            e_ps = pqpool.tile([B_LOC * SC, P], f32)
            nc.tensor.transpose(e_ps[:], E[:], identity[:])
            e_t = consts.tile([B_LOC * SC, P], f32)
            nc.scalar.copy(e_t[:], e_ps[:])
            e_rows = consts.tile([B_LOC, S], f32)
            nc.sync.dma_start(
                out=e_rows[:].rearrange("b (c i) -> b c i", i=P), in_=e_t[:]
            )

            neg_max = consts.tile([B_LOC, 1], f32)
            nc.vector.reduce_max(
                neg_max[:], e_rows[:], axis=mybir.AxisListType.X, negate=True
            )

            p_exp = consts.tile([B_LOC, S], f32)
            den = consts.tile([B_LOC, 1], f32)
            nc.scalar.activation(
                p_exp[:],
                e_rows[:],
                mybir.ActivationFunctionType.Exp,
                bias=neg_max[:],
                scale=1.0,
                accum_out=den[:],
            )

            recip = consts.tile([B_LOC, 1], f32)
            nc.vector.reciprocal(recip[:], den[:])

            a_t = consts.tile([B_LOC, S], f32)
            nc.vector.tensor_scalar_mul(a_t[:], p_exp[:], recip[:])

            nc.scalar.dma_start(out=attn, in_=a_t[:])

    nc.compile()
    return nc


def _get_nc():
    if "nc" not in _CACHE:
        _CACHE["nc"] = _build_bass()
    return _CACHE["nc"]


def _shard_inputs(questions, facts):
    questions = np.ascontiguousarray(np.asarray(questions), dtype=np.float32)
    facts = np.ascontiguousarray(np.asarray(facts), dtype=np.float32)
    in_maps = []
    for i in range(N_CORES):
        sl = slice(i * B_LOC, (i + 1) * B_LOC)
        in_maps.append(
            {
                "facts": np.ascontiguousarray(facts[sl]),
                "questions": np.ascontiguousarray(questions[sl]),
            }
        )
    return in_maps


def _run(questions, facts, **run_kwargs):
    from concourse.bass_utils import run_bass_kernel_spmd

    nc = _get_nc()
    in_maps = _shard_inputs(questions, facts)
    res = run_bass_kernel_spmd(nc, in_maps, core_ids=list(range(N_CORES)), **run_kwargs)
    out = np.stack([np.asarray(res.results[i]["attn"]) for i in range(N_CORES)])
    return out.reshape(B, S)[:, None, :].astype(np.float32), res


def kernel(questions, facts):
    out, _ = _run(questions, facts)
    return out


# revision 10
# speedup vs baseline: 1.1655x; 1.1655x over previous
"""AttnNet kernel for Trainium2: attn = softmax(einsum("bsh,bh->bs", facts, questions))[:, None, :].

Full shapes: questions [64, 4096] f32, facts [64, 512, 4096] f32 -> out [64, 1, 512] f32.
Data-parallel over batch: 8 batches per NeuronCore x 8 cores, no collectives.

Per-core dataflow (B_LOC=8, S=512, H=4096):
  - facts streamed as 32 contiguous [128(s), 4096(h)] tiles (2 MiB each), DMA
    instructions alternating between the two HWDGE rings (sync + scalar) so DGE
    setup on one ring overlaps transfers on the other (keeps all 16 DMA engines fed).
  - questions loaded once as a flat [1, 8*4096] tile on partition 0; q[b]
    broadcast to 128 partitions via gpsimd partition_broadcast per batch.
  - Fused DVE tensor_tensor_reduce: (facts_tile * q_b) with row-sum accumulate
    -> energies column E[:, b*4+c] in one pass (no separate ACT pass).
  - Epilogue: PE-transpose E [128,32] -> [32,128] (PSUM), ACT copy to SBUF,
    regroup to [8, 512] via SBUF->SBUF DMA, then softmax: -max (DVE), fused
    exp+sum (ACT), reciprocal + scale (DVE), DMA out.
"""

import numpy as np

B, S, H = 64, 512, 4096
N_CORES = 8
B_LOC = B // N_CORES  # 8
P = 128
SC = S // P  # 4 s-chunks per batch

_CACHE = {}


def _build_bass():
    import concourse.bacc as bacc
    import concourse.mybir as mybir
    import concourse.tile as tile
    from concourse.masks import make_identity

    f32 = mybir.dt.float32

    nc = bacc.Bacc("TRN2", target_bir_lowering=False, debug=False)
    facts = nc.dram_tensor("facts", [B_LOC, S, H], f32, kind="ExternalInput").ap()
    questions = nc.dram_tensor("questions", [B_LOC, H], f32, kind="ExternalInput").ap()
    attn = nc.dram_tensor("attn", [B_LOC, S], f32, kind="ExternalOutput").ap()

    with tile.TileContext(nc) as tc:
        with (
            tc.tile_pool(name="consts", bufs=1) as consts,
            tc.tile_pool(name="fpool", bufs=7) as fpool,
            tc.tile_pool(name="qrow", bufs=2) as qrow,
            tc.tile_pool(name="qsb", bufs=2) as qsb,
            tc.tile_pool(name="pq", bufs=1, space="PSUM") as pqpool,
        ):
            identity = consts.tile([P, P], f32)
            make_identity(nc, identity[:])



            # energies, column b*SC+c holds energies[b, c*128:(c+1)*128] on partitions
            E = consts.tile([P, B_LOC * SC], f32)

            qrings = [nc.sync, nc.scalar]
            for b in range(B_LOC):
                q_row = qrow.tile([1, H], f32)
                qrings[b % 2].dma_start(out=q_row[:], in_=questions[b : b + 1, :])
                q_b = qsb.tile([P, H], f32)
                nc.gpsimd.partition_broadcast(q_b[:], q_row[:])

                for c in range(SC):
                    ftile = fpool.tile([P, H], f32)
                    k = b * SC + c
                    qrings[k % 2].dma_start(
                        out=ftile[:], in_=facts[b, c * P : (c + 1) * P, :]
                    )
                    nc.vector.tensor_mul(out=ftile[:], in0=ftile[:], in1=q_b[:])
                    nc.scalar.activation(
                        ftile[:],
                        ftile[:],
                        mybir.ActivationFunctionType.Copy,
                        accum_out=E[:, k : k + 1],
                    )

            # --- softmax epilogue ---
            # transpose E [128, 32] -> [32, 128] (PSUM), copy to SBUF, then regroup
            # [32, 128] (p = b*4+c) -> [8, 512]: both traverse in (b, c, i) order
            e_ps = pqpool.tile([B_LOC * SC, P], f32)
            nc.tensor.transpose(e_ps[:], E[:], identity[:])
            e_t = consts.tile([B_LOC * SC, P], f32)
            nc.scalar.copy(e_t[:], e_ps[:])
            e_rows = consts.tile([B_LOC, S], f32)
            nc.sync.dma_start(
                out=e_rows[:].rearrange("b (c i) -> b c i", i=P), in_=e_t[:]
            )

            neg_max = consts.tile([B_LOC, 1], f32)
            nc.vector.reduce_max(
                neg_max[:], e_rows[:], axis=mybir.AxisListType.X, negate=True
            )

            p_exp = consts.tile([B_LOC, S], f32)
            den = consts.tile([B_LOC, 1], f32)
            nc.scalar.activation(
                p_exp[:],
                e_rows[:],
                mybir.ActivationFunctionType.Exp,
                bias=neg_max[:],
                scale=1.0,
                accum_out=den[:],
            )

            recip = consts.tile([B_LOC, 1], f32)
            nc.vector.reciprocal(recip[:], den[:])

            a_t = consts.tile([B_LOC, S], f32)
            nc.vector.tensor_scalar_mul(a_t[:], p_exp[:], recip[:])

            nc.scalar.dma_start(out=attn, in_=a_t[:])

    nc.compile()
    return nc


def _get_nc():
    if "nc" not in _CACHE:
        _CACHE["nc"] = _build_bass()
    return _CACHE["nc"]


def _shard_inputs(questions, facts):
    questions = np.ascontiguousarray(np.asarray(questions), dtype=np.float32)
    facts = np.ascontiguousarray(np.asarray(facts), dtype=np.float32)
    in_maps = []
    for i in range(N_CORES):
        sl = slice(i * B_LOC, (i + 1) * B_LOC)
        in_maps.append(
            {
                "facts": np.ascontiguousarray(facts[sl]),
                "questions": np.ascontiguousarray(questions[sl]),
            }
        )
    return in_maps


def _run(questions, facts, **run_kwargs):
    from concourse.bass_utils import run_bass_kernel_spmd

    nc = _get_nc()
    in_maps = _shard_inputs(questions, facts)
    res = run_bass_kernel_spmd(nc, in_maps, core_ids=list(range(N_CORES)), **run_kwargs)
    out = np.stack([np.asarray(res.results[i]["attn"]) for i in range(N_CORES)])
    return out.reshape(B, S)[:, None, :].astype(np.float32), res


def kernel(questions, facts):
    out, _ = _run(questions, facts)
    return out


# revision 11
# speedup vs baseline: 1.2246x; 1.0507x over previous
"""AttnNet kernel for Trainium2: attn = softmax(einsum("bsh,bh->bs", facts, questions))[:, None, :].

Full shapes: questions [64, 4096] f32, facts [64, 512, 4096] f32 -> out [64, 1, 512] f32.
Data-parallel over batch: 8 batches per NeuronCore x 8 cores, no collectives.

Per-core dataflow (B_LOC=8, S=512, H=4096):
  - facts streamed as 32 contiguous [128(s), 4096(h)] tiles (2 MiB each) at HBM line rate.
  - q[b] broadcast to 128 partitions via gpsimd partition_broadcast (otherwise-idle
    engine; costs ~8 us/batch and some DVE port contention, cheapest option measured).
  - DVE tensor_mul (facts_tile * q_b) then ACT activation(Copy, accum_out) row-sum
    -> energies column E[:, b*4+c] ([128,1] per s-chunk).
  - Epilogue: PE-transpose E [128,32] -> [32,128] (PSUM), ACT copy to SBUF, regroup
    to [8, 512] via SBUF->SBUF DMA, then softmax: -max (DVE), fused exp+sum (ACT),
    reciprocal + scale (DVE), DMA out.
"""

import numpy as np

B, S, H = 64, 512, 4096
N_CORES = 8
B_LOC = B // N_CORES  # 8
P = 128
SC = S // P  # 4 s-chunks per batch

_CACHE = {}


def _build_bass():
    import concourse.bacc as bacc
    import concourse.mybir as mybir
    import concourse.tile as tile
    from concourse.masks import make_identity

    f32 = mybir.dt.float32

    nc = bacc.Bacc("TRN2", target_bir_lowering=False, debug=False)
    facts = nc.dram_tensor("facts", [B_LOC, S, H], f32, kind="ExternalInput").ap()
    questions = nc.dram_tensor("questions", [B_LOC, H], f32, kind="ExternalInput").ap()
    attn = nc.dram_tensor("attn", [B_LOC, S], f32, kind="ExternalOutput").ap()

    with tile.TileContext(nc) as tc:
        with (
            tc.tile_pool(name="consts", bufs=1) as consts,
            tc.tile_pool(name="fpool", bufs=6) as fpool,
            tc.tile_pool(name="spool", bufs=2) as spool,
            tc.tile_pool(name="qrow", bufs=2) as qrow,
            tc.tile_pool(name="qsb", bufs=2) as qsb,
            tc.tile_pool(name="pq", bufs=1, space="PSUM") as pqpool,
        ):
            identity = consts.tile([P, P], f32)
            make_identity(nc, identity[:])

            # energies, column b*SC+c holds energies[b, c*128:(c+1)*128] on partitions
            E = consts.tile([P, B_LOC * SC], f32)

            for b in range(B_LOC):
                # q[b] to partition 0, then gpsimd broadcast to all 128 partitions
                q_row = qrow.tile([1, H], f32)
                # scalar HWDGE ring: don't queue behind the 2 MiB facts DMAs
                nc.scalar.dma_start(out=q_row[:], in_=questions[b : b + 1, :])
                q_b = qsb.tile([P, H], f32)
                nc.gpsimd.partition_broadcast(q_b[:], q_row[:])

                for c in range(SC):
                    ftile = fpool.tile([P, H], f32)
                    nc.sync.dma_start(
                        out=ftile[:], in_=facts[b, c * P : (c + 1) * P, :]
                    )
                    col = b * SC + c
                    # non-in-place multiply into alternating scratch (frees ftile
                    # after the DVE read; avoids read/write on the same SBUF tile)
                    prod = spool.tile([P, H], f32)
                    nc.vector.tensor_mul(out=prod[:], in0=ftile[:], in1=q_b[:])
                    nc.scalar.activation(
                        prod[:],
                        prod[:],
                        mybir.ActivationFunctionType.Copy,
                        accum_out=E[:, col : col + 1],
                    )

            # --- softmax epilogue ---
            # transpose E [128, 32] -> [32, 128] (PSUM), copy to SBUF, then regroup
            # [32, 128] (p = b*4+c) -> [8, 512]: both traverse in (b, c, i) order
            e_ps = pqpool.tile([B_LOC * SC, P], f32)
            nc.tensor.transpose(e_ps[:], E[:], identity[:])
            e_t = consts.tile([B_LOC * SC, P], f32)
            nc.scalar.copy(e_t[:], e_ps[:])
            e_rows = consts.tile([B_LOC, S], f32)
            nc.sync.dma_start(
                out=e_rows[:].rearrange("b (c i) -> b c i", i=P), in_=e_t[:]
            )

            neg_max = consts.tile([B_LOC, 1], f32)
            nc.vector.reduce_max(
                neg_max[:], e_rows[:], axis=mybir.AxisListType.X, negate=True
            )

            p_exp = consts.tile([B_LOC, S], f32)
            den = consts.tile([B_LOC, 1], f32)
            nc.scalar.activation(
                p_exp[:],
                e_rows[:],
                mybir.ActivationFunctionType.Exp,
                bias=neg_max[:],
                scale=1.0,
                accum_out=den[:],
            )

            recip = consts.tile([B_LOC, 1], f32)
            nc.vector.reciprocal(recip[:], den[:])

            a_t = consts.tile([B_LOC, S], f32)
            nc.vector.tensor_scalar_mul(a_t[:], p_exp[:], recip[:])

            nc.sync.dma_start(out=attn, in_=a_t[:])

    nc.compile()
    return nc


def _get_nc():
    if "nc" not in _CACHE:
        _CACHE["nc"] = _build_bass()
    return _CACHE["nc"]


def _shard_inputs(questions, facts):
    questions = np.ascontiguousarray(np.asarray(questions), dtype=np.float32)
    facts = np.ascontiguousarray(np.asarray(facts), dtype=np.float32)
    in_maps = []
    for i in range(N_CORES):
        sl = slice(i * B_LOC, (i + 1) * B_LOC)
        in_maps.append(
            {
                "facts": np.ascontiguousarray(facts[sl]),
                "questions": np.ascontiguousarray(questions[sl]),
            }
        )
    return in_maps


def _run(questions, facts, **run_kwargs):
    from concourse.bass_utils import run_bass_kernel_spmd

    nc = _get_nc()
    in_maps = _shard_inputs(questions, facts)
    res = run_bass_kernel_spmd(nc, in_maps, core_ids=list(range(N_CORES)), **run_kwargs)
    out = np.stack([np.asarray(res.results[i]["attn"]) for i in range(N_CORES)])
    return out.reshape(B, S)[:, None, :].astype(np.float32), res


def kernel(questions, facts):
    out, _ = _run(questions, facts)
    return out



# revision 13
# speedup vs baseline: 1.4391x; 1.1751x over previous
"""AttnNet kernel for Trainium2: attn = softmax(einsum("bsh,bh->bs", facts, questions))[:, None, :].

Full shapes: questions [64, 4096] f32, facts [64, 512, 4096] f32 -> out [64, 1, 512] f32.
Data-parallel over batch: 8 batches per NeuronCore x 8 cores, no collectives.

Per-core dataflow (B_LOC=8, S=512, H=4096):
  - facts streamed as 32 contiguous [128(s), 4096(h)] tiles (2 MiB each) on the
    sync HWDGE ring only (a second ring slows every descriptor and stalls the
    hosting engine's compute stream -- measured).
  - q[0] replicated to 128 partitions via a DMA broadcast read on the scalar
    ring (cuts the ~15 us gpsimd cold-start off the critical path); q[1..7]
    broadcast via gpsimd partition_broadcast (otherwise-idle engine).
  - DVE tensor_mul in place (fastest DVE mode: 2 SBUF streams, not 3) then ACT
    activation(Copy, accum_out) row-sum -> energies column E[:, b*4+c].
  - Two-phase softmax epilogue: batches 0-3 processed mid-stream (DMAs on the
    gpsimd SWDGE queue to avoid blocking the facts ring), batches 4-7 at the
    end (sync ring, which is empty by then). Each phase: PE-transpose E half
    [128,16] -> [16,128] (PSUM), ACT copy to SBUF, regroup to [4, 512] via
    SBUF->SBUF DMA, then softmax: -max (DVE), fused exp+sum (ACT),
    reciprocal + scale (DVE), DMA out.
"""

import numpy as np

B, S, H = 64, 512, 4096
N_CORES = 8
B_LOC = B // N_CORES  # 8
P = 128
SC = S // P  # 4 s-chunks per batch

_CACHE = {}


def _build_bass():
    import concourse.bacc as bacc
    import concourse.mybir as mybir
    import concourse.tile as tile
    from concourse.masks import make_identity

    f32 = mybir.dt.float32

    nc = bacc.Bacc("TRN2", target_bir_lowering=False, debug=False)
    facts = nc.dram_tensor("facts", [B_LOC, S, H], f32, kind="ExternalInput").ap()
    questions = nc.dram_tensor("questions", [B_LOC, H], f32, kind="ExternalInput").ap()
    attn = nc.dram_tensor("attn", [B_LOC, S], f32, kind="ExternalOutput").ap()

    with tile.TileContext(nc) as tc:
        with (
            tc.tile_pool(name="consts", bufs=1) as consts,
            tc.tile_pool(name="fpool", bufs=7) as fpool,
            tc.tile_pool(name="qrow", bufs=2) as qrow,
            tc.tile_pool(name="qsb", bufs=2) as qsb,
            tc.tile_pool(name="pq", bufs=2, space="PSUM") as pqpool,
        ):
            identity = consts.tile([P, P], f32)
            make_identity(nc, identity[:])

            # energies, column b*SC+c holds energies[b, c*128:(c+1)*128] on partitions
            E = consts.tile([P, B_LOC * SC], f32)

            for b in range(B_LOC):
                q_b = qsb.tile([P, H], f32)
                if b == 0:
                    # replicate q[0] across partitions straight from HBM on the
                    # scalar ring: ready long before the gpsimd Q7 cores warm up
                    nc.scalar.dma_start(
                        out=q_b[:], in_=questions[0:1, :].partition_broadcast(P)
                    )
                else:
                    q_row = qrow.tile([1, H], f32)
                    # scalar HWDGE ring: don't queue behind the 2 MiB facts DMAs
                    nc.scalar.dma_start(out=q_row[:], in_=questions[b : b + 1, :])
                    nc.gpsimd.partition_broadcast(q_b[:], q_row[:])

                for c in range(SC):
                    ftile = fpool.tile([P, H], f32)
                    nc.sync.dma_start(
                        out=ftile[:], in_=facts[b, c * P : (c + 1) * P, :]
                    )
                    col = b * SC + c
                    # in-place multiply (frees SBUF for a deeper DMA pipeline)
                    nc.vector.tensor_mul(out=ftile[:], in0=ftile[:], in1=q_b[:])
                    # ACT fused copy+accumulate (in place): accum_out = row sum
                    nc.scalar.activation(
                        ftile[:],
                        ftile[:],
                        mybir.ActivationFunctionType.Copy,
                        accum_out=E[:, col : col + 1],
                    )

                if b == B_LOC // 2 - 1:
                    _softmax_phase(nc, tc, mybir, consts, pqpool, identity, E, attn,
                                   0, B_LOC // 2, nc.gpsimd)

            _softmax_phase(nc, tc, mybir, consts, pqpool, identity, E, attn,
                           B_LOC // 2, B_LOC, nc.sync)

    nc.compile()
    return nc


def _softmax_phase(nc, tc, mybir, consts, pqpool, identity, E, attn, b0, b1, ring):
    """Transpose + softmax + store for batches [b0, b1)."""
    import concourse.mybir as mybir  # noqa: F811

    f32 = mybir.dt.float32
    nb = b1 - b0
    ncols = nb * SC

    # transpose E[:, b0*SC : b1*SC] [128, ncols] -> [ncols, 128] (PSUM)
    e_ps = pqpool.tile([ncols, P], f32)
    nc.tensor.transpose(e_ps[:], E[:, b0 * SC : b1 * SC], identity[:])
    e_t = consts.tile([ncols, P], f32)
    nc.scalar.copy(e_t[:], e_ps[:])
    # regroup [ncols, 128] (p = (b-b0)*4+c) -> [nb, 512]
    e_rows = consts.tile([nb, S], f32)
    ring.dma_start(out=e_rows[:].rearrange("b (c i) -> b c i", i=P), in_=e_t[:])

    neg_max = consts.tile([nb, 1], f32)
    nc.vector.reduce_max(
        neg_max[:], e_rows[:], axis=mybir.AxisListType.X, negate=True
    )

    p_exp = consts.tile([nb, S], f32)
    den = consts.tile([nb, 1], f32)
    nc.scalar.activation(
        p_exp[:],
        e_rows[:],
        mybir.ActivationFunctionType.Exp,
        bias=neg_max[:],
        scale=1.0,
        accum_out=den[:],
    )

    recip = consts.tile([nb, 1], f32)
    nc.vector.reciprocal(recip[:], den[:])

    a_t = consts.tile([nb, S], f32)
    nc.vector.tensor_scalar_mul(a_t[:], p_exp[:], recip[:])

    ring.dma_start(out=attn[b0:b1, :], in_=a_t[:])


def _get_nc():
    if "nc" not in _CACHE:
        _CACHE["nc"] = _build_bass()
    return _CACHE["nc"]


def _shard_inputs(questions, facts):
    questions = np.ascontiguousarray(np.asarray(questions), dtype=np.float32)
    facts = np.ascontiguousarray(np.asarray(facts), dtype=np.float32)
    in_maps = []
    for i in range(N_CORES):
        sl = slice(i * B_LOC, (i + 1) * B_LOC)
        in_maps.append(
            {
                "facts": np.ascontiguousarray(facts[sl]),
                "questions": np.ascontiguousarray(questions[sl]),
            }
        )
    return in_maps


def _run(questions, facts, **run_kwargs):
    from concourse.bass_utils import run_bass_kernel_spmd

    nc = _get_nc()
    in_maps = _shard_inputs(questions, facts)
    res = run_bass_kernel_spmd(nc, in_maps, core_ids=list(range(N_CORES)), **run_kwargs)
    out = np.stack([np.asarray(res.results[i]["attn"]) for i in range(N_CORES)])
    return out.reshape(B, S)[:, None, :].astype(np.float32), res


def kernel(questions, facts):
    out, _ = _run(questions, facts)
    return out
